# revision 20
# baseline (speedup 1.0000x reference)
"""LinearDeepSeekV3 fp8-blockquant GEMM for 8x TRN2 NeuronCores.

y = dequant_fp8_blockquant(x) @ dequant(w_q, w_s).T

Strategy (v2):
- Data-parallel: x rows [8192, 7168] split 1024/core; w replicated (host-prepacked
  to bf16 [K, N] = dequantized + transposed), loaded in kb-chunks interleaved
  with the first x tiles so quant can start immediately.
- Per core, per 128-row tile:
    DVE:    per-(row,128-block) absmax (1x reduce), scales, and HALF the
            quant-muls -> fp8 (scale absmax/224: TRN fp8e4 max=240 vs OCP
            e4m3fn 448; /2 grid shift is exact except denormals)
    GPSIMD: diag(s2) strips via affine_select + the other half of quant-muls
    PE:     transpose+dequant fused: xdT = matmul(lhsT=xq, rhs=diag(s2))
    ACT:    PSUM->SBUF copies (transpose results and y)
    PE:     main GEMM accumulating 56 k-blocks in PSUM
"""
import numpy as np
import ml_dtypes
from contextlib import ExitStack

import concourse.bass as bass
from concourse import bacc
import concourse.mybir as mybir
import concourse.tile as tile
from concourse import bass_utils

F32 = mybir.dt.float32
BF16 = mybir.dt.bfloat16
FP8 = mybir.dt.float8e4

P = 128
B, S, K, N = 2, 4096, 7168, 512
ROWS = B * S              # 8192
NCORES = 8
R_CORE = ROWS // NCORES   # 1024
RT = R_CORE // P          # 8 row tiles per core
KB = K // P               # 56 k-blocks
XCH = 4                   # x load/reduce chunks per row tile
KB_CH = KB // XCH         # 14 k-blocks per chunk
G4 = 4                    # transpose psum batch (4 k-blocks -> one 2KB bank)
WCH = 8                   # w load chunks
KB_W = KB // WCH          # 7 k-blocks per w chunk

_CACHE = {}


def build_nc(repeat=1):
    nc = bacc.Bacc("TRN2", debug=False)
    x_d = nc.dram_tensor("x", [R_CORE, K], F32, kind="ExternalInput")
    # host-prepacked partition-major: [P, KB*N] so the load is contiguous
    w_d = nc.dram_tensor("w", [P, KB * N], BF16, kind="ExternalInput")
    y_d = nc.dram_tensor("y", [R_CORE, N], F32, kind="ExternalOutput")

    with tile.TileContext(nc) as tc, ExitStack() as ctx:
        wp = ctx.enter_context(tc.tile_pool(name="wp", bufs=1))
        xp = ctx.enter_context(tc.tile_pool(name="xp", bufs=5))
        qp = ctx.enter_context(tc.tile_pool(name="qp", bufs=3))
        dp = ctx.enter_context(tc.tile_pool(name="dp", bufs=2))
        tp = ctx.enter_context(tc.tile_pool(name="tp", bufs=3))
        yp = ctx.enter_context(tc.tile_pool(name="yp", bufs=2))
        sp = ctx.enter_context(tc.tile_pool(name="sp", bufs=2))
        pst = ctx.enter_context(tc.tile_pool(name="pst", bufs=5, space="PSUM"))
        psy = ctx.enter_context(tc.tile_pool(name="psy", bufs=2, space="PSUM"))

        w = wp.tile([P, KB, N], BF16)
        wv = w_d.ap().rearrange("p (kb n) -> p kb n", kb=KB)
        xv = x_d.ap().rearrange("(rt p) k -> rt p k", p=P)
        yv = y_d.ap().rearrange("(rt p) n -> rt p n", p=P)

        # Prefetch x of tiles 0-4 with w chunks interleaved: x feeds the
        # quant engines (the long pole with DMA); w is only needed by the
        # main GEMM, which has slack, so it yields queue priority to x.
        xc_pre = {}

        def load_x(rt, c):
            xc = xp.tile([P, KB_CH * P], F32, tag="xc")
            nc.sync.dma_start(
                out=xc,
                in_=xv[rt, :, c * (KB_CH * P):(c + 1) * (KB_CH * P)])
            xc_pre[(rt, c)] = xc

        def load_w(wc):
            nc.sync.dma_start(
                out=w[:, wc * KB_W:(wc + 1) * KB_W, :],
                in_=wv[:, wc * KB_W:(wc + 1) * KB_W, :])

        for rt in range(2):
            for c in range(XCH):
                load_x(rt, c)
        for rt in range(2, 5):
            load_w(2 * (rt - 2)); load_w(2 * (rt - 2) + 1)
            for c in range(XCH):
                load_x(rt, c)
        load_w(6); load_w(7)

        for rep in range(repeat):
          for rt in range(RT):
            # ---- quantize pipeline (chunked along K) ----
            # separate quant output tiles per writer engine (avoids
            # concurrent-writer SBUF stalls on a shared tile)
            xqa = qp.tile([P, XCH, KB_CH // 2, P], FP8, tag="xqa")
            xqb = qp.tile([P, XCH, KB_CH // 2, P], FP8, tag="xqb")
            am = sp.tile([P, KB], F32, tag="am")
            rc = sp.tile([P, KB], F32, tag="rc")
            r2 = sp.tile([P, KB], F32, tag="r2")
            s2b = sp.tile([P, KB], BF16, tag="s2b")
            diag = dp.tile([P, KB, P], BF16, tag="diag")
            for c in range(XCH):
                if rep == 0 and (rt, c) in xc_pre:
                    xc = xc_pre[(rt, c)]
                else:
                    xc = xp.tile([P, KB_CH * P], F32, tag="xc")
                    nc.sync.dma_start(
                        out=xc,
                        in_=xv[rt, :, c * (KB_CH * P):(c + 1) * (KB_CH * P)])
                xc3 = xc.rearrange("p (kb j) -> p kb j", j=P)
                sl = slice(c * KB_CH, (c + 1) * KB_CH)
                nc.vector.tensor_reduce(
                    out=am[:, sl], in_=xc3, axis=mybir.AxisListType.X,
                    op=mybir.AluOpType.max, apply_absolute_value=True)
                nc.vector.reciprocal(out=rc[:, sl], in_=am[:, sl])
                # s2b = bf16(am/224), r2 = 224/am on ACT; diag strip per chunk
                nc.scalar.activation(
                    out=s2b[:, sl], in_=am[:, sl],
                    func=mybir.ActivationFunctionType.Copy, scale=1.0 / 224.0)
                nc.scalar.activation(
                    out=r2[:, sl], in_=rc[:, sl],
                    func=mybir.ActivationFunctionType.Copy, scale=224.0)
                nc.gpsimd.affine_select(
                    out=diag[:, sl, :],
                    in_=s2b[:, sl, None].broadcast_to([P, KB_CH, P]),
                    compare_op=mybir.AluOpType.is_equal,
                    fill=0.0, base=0,
                    pattern=[[0, KB_CH], [-1, P]],
                    channel_multiplier=1)
                # quant-mul xq = x * (224/am), split DVE / GpSimd
                for h in range(2):
                    hs = slice(h * (KB_CH // 2), (h + 1) * (KB_CH // 2))
                    ksl = slice(c * KB_CH + h * (KB_CH // 2),
                                c * KB_CH + (h + 1) * (KB_CH // 2))
                    if h == 0:
                        nc.vector.scalar_tensor_tensor(
                            out=xqa[:, c],
                            in0=xc3[:, hs, :],
                            scalar=224.0,
                            in1=rc[:, ksl, None].broadcast_to(
                                [P, KB_CH // 2, P]),
                            op0=mybir.AluOpType.mult,
                            op1=mybir.AluOpType.mult)
                    else:
                        nc.gpsimd.tensor_tensor(
                            out=xqb[:, c],
                            in0=xc3[:, hs, :],
                            in1=r2[:, ksl, None].broadcast_to(
                                [P, KB_CH // 2, P]),
                            op=mybir.AluOpType.mult)

            # ---- transpose+dequant on PE, copy out per 4-block group ----
            xdt = tp.tile([P, KB, P], BF16, tag="xdt")
            for g in range(KB // G4):
                pt = pst.tile([P, G4, P], F32, tag="pt")
                for i in range(G4):
                    kb = g * G4 + i
                    c, j = kb // KB_CH, kb % KB_CH
                    lhs = (xqa[:, c, j, :] if j < KB_CH // 2
                           else xqb[:, c, j - KB_CH // 2, :])
                    nc.tensor.matmul(pt[:, i, :], lhsT=lhs,
                                     rhs=diag[:, kb, :], start=True, stop=True)
                nc.scalar.activation(
                    out=xdt[:, g * G4:(g + 1) * G4, :], in_=pt,
                    func=mybir.ActivationFunctionType.Copy)

            # ---- main GEMM: accumulate 56 k-blocks ----
            py = psy.tile([P, N], F32, tag="py")
            for kb in range(KB):
                nc.tensor.matmul(py, lhsT=xdt[:, kb, :], rhs=w[:, kb, :],
                                 start=(kb == 0), stop=(kb == KB - 1))
            y_sb = yp.tile([P, N], F32, tag="ysb")
            nc.scalar.activation(
                out=y_sb, in_=py, func=mybir.ActivationFunctionType.Copy)
            nc.sync.dma_start(out=yv[rt], in_=y_sb)

    nc.compile()
    return nc


def prep_w(w_q: np.ndarray, w_s: np.ndarray) -> np.ndarray:
    Nb, Kb = w_s.shape
    w_dq = (w_q.reshape(Nb, P, Kb, P) * w_s[:, None, :, None]).reshape(N, K)
    w_t = np.ascontiguousarray(w_dq.T).astype(ml_dtypes.bfloat16)  # [K, N]
    # partition-major repack: [P, KB*N] with w_pk[p, kb, n] = w_t[kb*128+p, n]
    return np.ascontiguousarray(
        w_t.reshape(Kb, P, N).transpose(1, 0, 2).reshape(P, Kb * N))


def kernel(x: np.ndarray, w_q: np.ndarray, w_s: np.ndarray, _trace=False) -> np.ndarray:
    if "nc" not in _CACHE:
        _CACHE["nc"] = build_nc()
    nc = _CACHE["nc"]

    w_host = prep_w(np.asarray(w_q, np.float32), np.asarray(w_s, np.float32))
    xf = np.ascontiguousarray(np.asarray(x, np.float32).reshape(ROWS, K))
    in_maps = [
        {"x": np.ascontiguousarray(xf[c * R_CORE:(c + 1) * R_CORE]), "w": w_host}
        for c in range(NCORES)
    ]
    res = bass_utils.run_bass_kernel_spmd(
        nc, in_maps, core_ids=list(range(NCORES)), trace=_trace)
    y = np.concatenate([res.results[c]["y"] for c in range(NCORES)], axis=0)
    _CACHE["last_exec_time_ns"] = res.exec_time_ns
    return y.reshape(B, S, N)


# revision 21
# speedup vs baseline: 1.0423x; 1.0423x over previous
"""LinearDeepSeekV3 fp8-blockquant GEMM for 8x TRN2 NeuronCores.

y = dequant_fp8_blockquant(x) @ dequant(w_q, w_s).T

Strategy (v2):
- Data-parallel: x rows [8192, 7168] split 1024/core; w replicated (host-prepacked
  to bf16 [K, N] = dequantized + transposed), loaded in kb-chunks interleaved
  with the first x tiles so quant can start immediately.
- Per core, per 128-row tile:
    DVE:    per-(row,128-block) absmax (1x reduce), scales, and HALF the
            quant-muls -> fp8 (scale absmax/224: TRN fp8e4 max=240 vs OCP
            e4m3fn 448; /2 grid shift is exact except denormals)
    GPSIMD: diag(s2) strips via affine_select + the other half of quant-muls
    PE:     transpose+dequant fused: xdT = matmul(lhsT=xq, rhs=diag(s2))
    ACT:    PSUM->SBUF copies (transpose results and y)
    PE:     main GEMM accumulating 56 k-blocks in PSUM
"""
import numpy as np
import ml_dtypes
from contextlib import ExitStack

import concourse.bass as bass
from concourse import bacc
import concourse.mybir as mybir
import concourse.tile as tile
from concourse import bass_utils

F32 = mybir.dt.float32
BF16 = mybir.dt.bfloat16
FP8 = mybir.dt.float8e4

P = 128
B, S, K, N = 2, 4096, 7168, 512
ROWS = B * S              # 8192
NCORES = 8
R_CORE = ROWS // NCORES   # 1024
RT = R_CORE // P          # 8 row tiles per core
KB = K // P               # 56 k-blocks
XCH = 4                   # x load/reduce chunks per row tile
KB_CH = KB // XCH         # 14 k-blocks per chunk
G4 = 4                    # transpose psum batch (4 k-blocks -> one 2KB bank)
WCH = 8                   # w load chunks
KB_W = KB // WCH          # 7 k-blocks per w chunk

_CACHE = {}


def build_nc(repeat=1):
    nc = bacc.Bacc("TRN2", debug=False)
    x_d = nc.dram_tensor("x", [R_CORE, K], F32, kind="ExternalInput")
    # host-prepacked partition-major: [P, KB*N] so the load is contiguous
    w_d = nc.dram_tensor("w", [P, KB * N], BF16, kind="ExternalInput")
    y_d = nc.dram_tensor("y", [R_CORE, N], F32, kind="ExternalOutput")

    with tile.TileContext(nc) as tc, ExitStack() as ctx:
        wp = ctx.enter_context(tc.tile_pool(name="wp", bufs=1))
        xp = ctx.enter_context(tc.tile_pool(name="xp", bufs=5))
        qp = ctx.enter_context(tc.tile_pool(name="qp", bufs=3))
        dp = ctx.enter_context(tc.tile_pool(name="dp", bufs=2))
        tp = ctx.enter_context(tc.tile_pool(name="tp", bufs=3))
        yp = ctx.enter_context(tc.tile_pool(name="yp", bufs=2))
        sp = ctx.enter_context(tc.tile_pool(name="sp", bufs=2))
        pst = ctx.enter_context(tc.tile_pool(name="pst", bufs=5, space="PSUM"))
        psy = ctx.enter_context(tc.tile_pool(name="psy", bufs=2, space="PSUM"))

        w = wp.tile([P, KB, N], BF16)
        wv = w_d.ap().rearrange("p (kb n) -> p kb n", kb=KB)
        xv = x_d.ap().rearrange("(rt p) k -> rt p k", p=P)
        yv = y_d.ap().rearrange("(rt p) n -> rt p n", p=P)

        # Prefetch x of tiles 0-4 with w chunks interleaved: x feeds the
        # quant engines (the long pole with DMA); w is only needed by the
        # main GEMM, which has slack, so it yields queue priority to x.
        xc_pre = {}

        def load_x(rt, c):
            xc = xp.tile([P, KB_CH * P], F32, tag="xc")
            nc.sync.dma_start(
                out=xc,
                in_=xv[rt, :, c * (KB_CH * P):(c + 1) * (KB_CH * P)])
            xc_pre[(rt, c)] = xc

        def load_w(wc):
            nc.sync.dma_start(
                out=w[:, wc * KB_W:(wc + 1) * KB_W, :],
                in_=wv[:, wc * KB_W:(wc + 1) * KB_W, :])

        for rt in range(2):
            for c in range(XCH):
                load_x(rt, c)
        for wc in range(WCH):
            load_w(wc)

        for rep in range(repeat):
          for rt in range(RT):
            # ---- quantize pipeline (chunked along K) ----
            # separate quant output tiles per writer engine (avoids
            # concurrent-writer SBUF stalls on a shared tile)
            xqa = qp.tile([P, XCH, KB_CH // 2, P], FP8, tag="xqa")
            xqb = qp.tile([P, XCH, KB_CH // 2, P], FP8, tag="xqb")
            am = sp.tile([P, KB], F32, tag="am")
            rc = sp.tile([P, KB], F32, tag="rc")
            r2 = sp.tile([P, KB], F32, tag="r2")
            s2b = sp.tile([P, KB], BF16, tag="s2b")
            diag = dp.tile([P, KB, P], BF16, tag="diag")
            for c in range(XCH):
                if rep == 0 and (rt, c) in xc_pre:
                    xc = xc_pre[(rt, c)]
                else:
                    xc = xp.tile([P, KB_CH * P], F32, tag="xc")
                    nc.sync.dma_start(
                        out=xc,
                        in_=xv[rt, :, c * (KB_CH * P):(c + 1) * (KB_CH * P)])
                xc3 = xc.rearrange("p (kb j) -> p kb j", j=P)
                sl = slice(c * KB_CH, (c + 1) * KB_CH)
                nc.vector.tensor_reduce(
                    out=am[:, sl], in_=xc3, axis=mybir.AxisListType.X,
                    op=mybir.AluOpType.max, apply_absolute_value=True)
                nc.vector.reciprocal(out=rc[:, sl], in_=am[:, sl])
                # s2b = bf16(am/224), r2 = 224/am on ACT; diag strip per chunk
                nc.scalar.activation(
                    out=s2b[:, sl], in_=am[:, sl],
                    func=mybir.ActivationFunctionType.Copy, scale=1.0 / 224.0)
                nc.scalar.activation(
                    out=r2[:, sl], in_=rc[:, sl],
                    func=mybir.ActivationFunctionType.Copy, scale=224.0)
                nc.gpsimd.affine_select(
                    out=diag[:, sl, :],
                    in_=s2b[:, sl, None].broadcast_to([P, KB_CH, P]),
                    compare_op=mybir.AluOpType.is_equal,
                    fill=0.0, base=0,
                    pattern=[[0, KB_CH], [-1, P]],
                    channel_multiplier=1)
                # quant-mul xq = x * (224/am), split DVE / GpSimd
                for h in range(2):
                    hs = slice(h * (KB_CH // 2), (h + 1) * (KB_CH // 2))
                    ksl = slice(c * KB_CH + h * (KB_CH // 2),
                                c * KB_CH + (h + 1) * (KB_CH // 2))
                    if h == 0:
                        nc.vector.scalar_tensor_tensor(
                            out=xqa[:, c],
                            in0=xc3[:, hs, :],
                            scalar=224.0,
                            in1=rc[:, ksl, None].broadcast_to(
                                [P, KB_CH // 2, P]),
                            op0=mybir.AluOpType.mult,
                            op1=mybir.AluOpType.mult)
                    else:
                        nc.gpsimd.tensor_tensor(
                            out=xqb[:, c],
                            in0=xc3[:, hs, :],
                            in1=r2[:, ksl, None].broadcast_to(
                                [P, KB_CH // 2, P]),
                            op=mybir.AluOpType.mult)

            # ---- transpose+dequant on PE, copy out per 4-block group ----
            xdt = tp.tile([P, KB, P], BF16, tag="xdt")
            for g in range(KB // G4):
                pt = pst.tile([P, G4, P], F32, tag="pt")
                for i in range(G4):
                    kb = g * G4 + i
                    c, j = kb // KB_CH, kb % KB_CH
                    lhs = (xqa[:, c, j, :] if j < KB_CH // 2
                           else xqb[:, c, j - KB_CH // 2, :])
                    nc.tensor.matmul(pt[:, i, :], lhsT=lhs,
                                     rhs=diag[:, kb, :], start=True, stop=True)
                nc.scalar.activation(
                    out=xdt[:, g * G4:(g + 1) * G4, :], in_=pt,
                    func=mybir.ActivationFunctionType.Copy)

            # ---- main GEMM: accumulate 56 k-blocks ----
            py = psy.tile([P, N], F32, tag="py")
            for kb in range(KB):
                nc.tensor.matmul(py, lhsT=xdt[:, kb, :], rhs=w[:, kb, :],
                                 start=(kb == 0), stop=(kb == KB - 1))
            y_sb = yp.tile([P, N], F32, tag="ysb")
            nc.scalar.activation(
                out=y_sb, in_=py, func=mybir.ActivationFunctionType.Copy)
            nc.sync.dma_start(out=yv[rt], in_=y_sb)

    nc.compile()
    return nc


def prep_w(w_q: np.ndarray, w_s: np.ndarray) -> np.ndarray:
    Nb, Kb = w_s.shape
    w_dq = (w_q.reshape(Nb, P, Kb, P) * w_s[:, None, :, None]).reshape(N, K)
    w_t = np.ascontiguousarray(w_dq.T).astype(ml_dtypes.bfloat16)  # [K, N]
    # partition-major repack: [P, KB*N] with w_pk[p, kb, n] = w_t[kb*128+p, n]
    return np.ascontiguousarray(
        w_t.reshape(Kb, P, N).transpose(1, 0, 2).reshape(P, Kb * N))


def kernel(x: np.ndarray, w_q: np.ndarray, w_s: np.ndarray, _trace=False) -> np.ndarray:
    if "nc" not in _CACHE:
        _CACHE["nc"] = build_nc()
    nc = _CACHE["nc"]

    w_host = prep_w(np.asarray(w_q, np.float32), np.asarray(w_s, np.float32))
    xf = np.ascontiguousarray(np.asarray(x, np.float32).reshape(ROWS, K))
    in_maps = [
        {"x": np.ascontiguousarray(xf[c * R_CORE:(c + 1) * R_CORE]), "w": w_host}
        for c in range(NCORES)
    ]
    res = bass_utils.run_bass_kernel_spmd(
        nc, in_maps, core_ids=list(range(NCORES)), trace=_trace)
    y = np.concatenate([res.results[c]["y"] for c in range(NCORES)], axis=0)
    _CACHE["last_exec_time_ns"] = res.exec_time_ns
    return y.reshape(B, S, N)


# revision 23
# speedup vs baseline: 1.0473x; 1.0048x over previous
"""LinearDeepSeekV3 fp8-blockquant GEMM for 8x TRN2 NeuronCores.

y = dequant_fp8_blockquant(x) @ dequant(w_q, w_s).T

Strategy (v2):
- Data-parallel: x rows [8192, 7168] split 1024/core; w replicated (host-prepacked
  to bf16 [K, N] = dequantized + transposed), loaded in kb-chunks interleaved
  with the first x tiles so quant can start immediately.
- Per core, per 128-row tile:
    DVE:    per-(row,128-block) absmax (1x reduce), scales, and HALF the
            quant-muls -> fp8 (scale absmax/224: TRN fp8e4 max=240 vs OCP
            e4m3fn 448; /2 grid shift is exact except denormals)
    GPSIMD: diag(s2) strips via affine_select + the other half of quant-muls
    PE:     transpose+dequant fused: xdT = matmul(lhsT=xq, rhs=diag(s2))
    ACT:    PSUM->SBUF copies (transpose results and y)
    PE:     main GEMM accumulating 56 k-blocks in PSUM
"""
import numpy as np
import ml_dtypes
from contextlib import ExitStack

import concourse.bass as bass
from concourse import bacc
import concourse.mybir as mybir
import concourse.tile as tile
from concourse import bass_utils

F32 = mybir.dt.float32
BF16 = mybir.dt.bfloat16
FP8 = mybir.dt.float8e4

P = 128
B, S, K, N = 2, 4096, 7168, 512
ROWS = B * S              # 8192
NCORES = 8
R_CORE = ROWS // NCORES   # 1024
RT = R_CORE // P          # 8 row tiles per core
KB = K // P               # 56 k-blocks
XCH = 4                   # x load/reduce chunks per row tile
KB_CH = KB // XCH         # 14 k-blocks per chunk
G4 = 4                    # transpose psum batch (4 k-blocks -> one 2KB bank)
WCH = 8                   # w load chunks
KB_W = KB // WCH          # 7 k-blocks per w chunk

_CACHE = {}


def build_nc(repeat=1):
    nc = bacc.Bacc("TRN2", debug=False)
    x_d = nc.dram_tensor("x", [R_CORE, K], F32, kind="ExternalInput")
    # host-prepacked partition-major: [P, KB*N] so the load is contiguous
    w_d = nc.dram_tensor("w", [P, KB * N], BF16, kind="ExternalInput")
    y_d = nc.dram_tensor("y", [R_CORE, N], F32, kind="ExternalOutput")

    with tile.TileContext(nc) as tc, ExitStack() as ctx:
        wp = ctx.enter_context(tc.tile_pool(name="wp", bufs=1))
        xp = ctx.enter_context(tc.tile_pool(name="xp", bufs=6))
        qp = ctx.enter_context(tc.tile_pool(name="qp", bufs=3))
        dp = ctx.enter_context(tc.tile_pool(name="dp", bufs=2))
        tp = ctx.enter_context(tc.tile_pool(name="tp", bufs=3))
        yp = ctx.enter_context(tc.tile_pool(name="yp", bufs=2))
        sp = ctx.enter_context(tc.tile_pool(name="sp", bufs=2))
        pst = ctx.enter_context(tc.tile_pool(name="pst", bufs=5, space="PSUM"))
        psy = ctx.enter_context(tc.tile_pool(name="psy", bufs=3, space="PSUM"))

        w = wp.tile([P, KB, N], BF16)
        wv = w_d.ap().rearrange("p (kb n) -> p kb n", kb=KB)
        xv = x_d.ap().rearrange("(rt p) k -> rt p k", p=P)
        yv = y_d.ap().rearrange("(rt p) n -> rt p n", p=P)

        # Prefetch x of tiles 0-4 with w chunks interleaved: x feeds the
        # quant engines (the long pole with DMA); w is only needed by the
        # main GEMM, which has slack, so it yields queue priority to x.
        xc_pre = {}

        def load_x(rt, c):
            xc = xp.tile([P, KB_CH * P], F32, tag="xc")
            nc.sync.dma_start(
                out=xc,
                in_=xv[rt, :, c * (KB_CH * P):(c + 1) * (KB_CH * P)])
            xc_pre[(rt, c)] = xc

        def load_w(wc):
            nc.sync.dma_start(
                out=w[:, wc * KB_W:(wc + 1) * KB_W, :],
                in_=wv[:, wc * KB_W:(wc + 1) * KB_W, :])

        for rt in range(2):
            for c in range(XCH):
                load_x(rt, c)
        for wc in range(WCH):
            load_w(wc)

        for rep in range(repeat):
          for rt in range(RT):
            # ---- quantize pipeline (chunked along K) ----
            # separate quant output tiles per writer engine (avoids
            # concurrent-writer SBUF stalls on a shared tile)
            xqa = qp.tile([P, XCH, KB_CH // 2, P], FP8, tag="xqa")
            xqb = qp.tile([P, XCH, KB_CH // 2, P], FP8, tag="xqb")
            am = sp.tile([P, KB], F32, tag="am")
            rc = sp.tile([P, KB], F32, tag="rc")
            r2 = sp.tile([P, KB], F32, tag="r2")
            s2b = sp.tile([P, KB], BF16, tag="s2b")
            diag = dp.tile([P, KB, P], BF16, tag="diag")
            for c in range(XCH):
                if rep == 0 and (rt, c) in xc_pre:
                    xc = xc_pre[(rt, c)]
                else:
                    xc = xp.tile([P, KB_CH * P], F32, tag="xc")
                    nc.sync.dma_start(
                        out=xc,
                        in_=xv[rt, :, c * (KB_CH * P):(c + 1) * (KB_CH * P)])
                xc3 = xc.rearrange("p (kb j) -> p kb j", j=P)
                sl = slice(c * KB_CH, (c + 1) * KB_CH)
                nc.vector.tensor_reduce(
                    out=am[:, sl], in_=xc3, axis=mybir.AxisListType.X,
                    op=mybir.AluOpType.max, apply_absolute_value=True)
                nc.vector.reciprocal(out=rc[:, sl], in_=am[:, sl])
                # s2b = bf16(am/224), r2 = 224/am on ACT; diag strip per chunk
                nc.scalar.activation(
                    out=s2b[:, sl], in_=am[:, sl],
                    func=mybir.ActivationFunctionType.Copy, scale=1.0 / 224.0)
                nc.scalar.activation(
                    out=r2[:, sl], in_=rc[:, sl],
                    func=mybir.ActivationFunctionType.Copy, scale=224.0)
                nc.gpsimd.affine_select(
                    out=diag[:, sl, :],
                    in_=s2b[:, sl, None].broadcast_to([P, KB_CH, P]),
                    compare_op=mybir.AluOpType.is_equal,
                    fill=0.0, base=0,
                    pattern=[[0, KB_CH], [-1, P]],
                    channel_multiplier=1)
                # quant-mul xq = x * (224/am), split DVE / GpSimd
                for h in range(2):
                    hs = slice(h * (KB_CH // 2), (h + 1) * (KB_CH // 2))
                    ksl = slice(c * KB_CH + h * (KB_CH // 2),
                                c * KB_CH + (h + 1) * (KB_CH // 2))
                    if h == 0:
                        nc.vector.scalar_tensor_tensor(
                            out=xqa[:, c],
                            in0=xc3[:, hs, :],
                            scalar=224.0,
                            in1=rc[:, ksl, None].broadcast_to(
                                [P, KB_CH // 2, P]),
                            op0=mybir.AluOpType.mult,
                            op1=mybir.AluOpType.mult)
                    else:
                        nc.gpsimd.tensor_tensor(
                            out=xqb[:, c],
                            in0=xc3[:, hs, :],
                            in1=r2[:, ksl, None].broadcast_to(
                                [P, KB_CH // 2, P]),
                            op=mybir.AluOpType.mult)

            # ---- transpose+dequant on PE, copy out per 4-block group ----
            xdt = tp.tile([P, KB, P], BF16, tag="xdt")
            for g in range(KB // G4):
                pt = pst.tile([P, G4, P], F32, tag="pt")
                for i in range(G4):
                    kb = g * G4 + i
                    c, j = kb // KB_CH, kb % KB_CH
                    lhs = (xqa[:, c, j, :] if j < KB_CH // 2
                           else xqb[:, c, j - KB_CH // 2, :])
                    nc.tensor.matmul(pt[:, i, :], lhsT=lhs,
                                     rhs=diag[:, kb, :], start=True, stop=True)
                nc.scalar.activation(
                    out=xdt[:, g * G4:(g + 1) * G4, :], in_=pt,
                    func=mybir.ActivationFunctionType.Copy)

            # ---- main GEMM: accumulate 56 k-blocks ----
            py = psy.tile([P, N], F32, tag="py")
            for kb in range(KB):
                nc.tensor.matmul(py, lhsT=xdt[:, kb, :], rhs=w[:, kb, :],
                                 start=(kb == 0), stop=(kb == KB - 1))
            y_sb = yp.tile([P, N], F32, tag="ysb")
            nc.scalar.activation(
                out=y_sb, in_=py, func=mybir.ActivationFunctionType.Copy)
            nc.sync.dma_start(out=yv[rt], in_=y_sb)

    nc.compile()
    return nc


def prep_w(w_q: np.ndarray, w_s: np.ndarray) -> np.ndarray:
    Nb, Kb = w_s.shape
    w_dq = (w_q.reshape(Nb, P, Kb, P) * w_s[:, None, :, None]).reshape(N, K)
    w_t = np.ascontiguousarray(w_dq.T).astype(ml_dtypes.bfloat16)  # [K, N]
    # partition-major repack: [P, KB*N] with w_pk[p, kb, n] = w_t[kb*128+p, n]
    return np.ascontiguousarray(
        w_t.reshape(Kb, P, N).transpose(1, 0, 2).reshape(P, Kb * N))


def kernel(x: np.ndarray, w_q: np.ndarray, w_s: np.ndarray, _trace=False) -> np.ndarray:
    if "nc" not in _CACHE:
        _CACHE["nc"] = build_nc()
    nc = _CACHE["nc"]

    w_host = prep_w(np.asarray(w_q, np.float32), np.asarray(w_s, np.float32))
    xf = np.ascontiguousarray(np.asarray(x, np.float32).reshape(ROWS, K))
    in_maps = [
        {"x": np.ascontiguousarray(xf[c * R_CORE:(c + 1) * R_CORE]), "w": w_host}
        for c in range(NCORES)
    ]
    res = bass_utils.run_bass_kernel_spmd(
        nc, in_maps, core_ids=list(range(NCORES)), trace=_trace)
    y = np.concatenate([res.results[c]["y"] for c in range(NCORES)], axis=0)
    _CACHE["last_exec_time_ns"] = res.exec_time_ns
    return y.reshape(B, S, N)


# revision 26
# speedup vs baseline: 1.0514x; 1.0038x over previous
"""LinearDeepSeekV3 fp8-blockquant GEMM for 8x TRN2 NeuronCores.

y = dequant_fp8_blockquant(x) @ dequant(w_q, w_s).T

Strategy (v2):
- Data-parallel: x rows [8192, 7168] split 1024/core; w replicated (host-prepacked
  to bf16 [K, N] = dequantized + transposed), loaded in kb-chunks interleaved
  with the first x tiles so quant can start immediately.
- Per core, per 128-row tile:
    DVE:    per-(row,128-block) absmax (1x reduce), scales, and HALF the
            quant-muls -> fp8 (scale absmax/224: TRN fp8e4 max=240 vs OCP
            e4m3fn 448; /2 grid shift is exact except denormals)
    GPSIMD: diag(s2) strips via affine_select + the other half of quant-muls
    PE:     transpose+dequant fused: xdT = matmul(lhsT=xq, rhs=diag(s2))
    ACT:    PSUM->SBUF copies (transpose results and y)
    PE:     main GEMM accumulating 56 k-blocks in PSUM
"""
import numpy as np
import ml_dtypes
from contextlib import ExitStack

import concourse.bass as bass
from concourse import bacc
import concourse.mybir as mybir
import concourse.tile as tile
from concourse import bass_utils

F32 = mybir.dt.float32
BF16 = mybir.dt.bfloat16
FP8 = mybir.dt.float8e4

P = 128
B, S, K, N = 2, 4096, 7168, 512
ROWS = B * S              # 8192
NCORES = 8
R_CORE = ROWS // NCORES   # 1024
RT = R_CORE // P          # 8 row tiles per core
KB = K // P               # 56 k-blocks
XCH = 4                   # x load/reduce chunks per row tile
KB_CH = KB // XCH         # 14 k-blocks per chunk
G4 = 4                    # transpose psum batch (4 k-blocks -> one 2KB bank)
WCH = 8                   # w load chunks
KB_W = KB // WCH          # 7 k-blocks per w chunk

_CACHE = {}


def build_nc(repeat=1):
    nc = bacc.Bacc("TRN2", debug=False)
    x_d = nc.dram_tensor("x", [R_CORE, K], F32, kind="ExternalInput")
    # host-prepacked partition-major: [P, KB*N] so the load is contiguous
    w_d = nc.dram_tensor("w", [P, KB * N], BF16, kind="ExternalInput")
    y_d = nc.dram_tensor("y", [R_CORE, N], F32, kind="ExternalOutput")

    with tile.TileContext(nc) as tc, ExitStack() as ctx:
        wp = ctx.enter_context(tc.tile_pool(name="wp", bufs=1))
        xp = ctx.enter_context(tc.tile_pool(name="xp", bufs=6))
        qp = ctx.enter_context(tc.tile_pool(name="qp", bufs=3))
        dp = ctx.enter_context(tc.tile_pool(name="dp", bufs=2))
        tp = ctx.enter_context(tc.tile_pool(name="tp", bufs=3))
        yp = ctx.enter_context(tc.tile_pool(name="yp", bufs=2))
        sp = ctx.enter_context(tc.tile_pool(name="sp", bufs=2))
        pst = ctx.enter_context(tc.tile_pool(name="pst", bufs=5, space="PSUM"))
        psy = ctx.enter_context(tc.tile_pool(name="psy", bufs=3, space="PSUM"))

        w = wp.tile([P, KB, N], BF16)
        wv = w_d.ap().rearrange("p (kb n) -> p kb n", kb=KB)
        xv = x_d.ap().rearrange("(rt p) k -> rt p k", p=P)
        yv = y_d.ap().rearrange("(rt p) n -> rt p n", p=P)

        # Prefetch x of tiles 0-4 with w chunks interleaved: x feeds the
        # quant engines (the long pole with DMA); w is only needed by the
        # main GEMM, which has slack, so it yields queue priority to x.
        xc_pre = {}

        def load_x(rt, c):
            xc = xp.tile([P, KB_CH * P], F32, tag="xc")
            nc.sync.dma_start(
                out=xc,
                in_=xv[rt, :, c * (KB_CH * P):(c + 1) * (KB_CH * P)])
            xc_pre[(rt, c)] = xc

        def load_w(wc):
            nc.sync.dma_start(
                out=w[:, wc * KB_W:(wc + 1) * KB_W, :],
                in_=wv[:, wc * KB_W:(wc + 1) * KB_W, :])

        for rt in range(2):
            for c in range(XCH):
                load_x(rt, c)
        for wc in range(WCH):
            load_w(wc)

        for rep in range(repeat):
          for rt in range(RT):
            # ---- quantize pipeline (chunked along K) ----
            # separate quant output tiles per writer engine (avoids
            # concurrent-writer SBUF stalls on a shared tile)
            xqa = qp.tile([P, XCH, KB_CH // 2, P], FP8, tag="xqa")
            xqb = qp.tile([P, XCH, KB_CH // 2, P], FP8, tag="xqb")
            am = sp.tile([P, KB], F32, tag="am")
            rc = sp.tile([P, KB], F32, tag="rc")
            r2 = sp.tile([P, KB], F32, tag="r2")
            s2b = sp.tile([P, KB], BF16, tag="s2b")
            diag = dp.tile([P, KB, P], BF16, tag="diag")
            for c in range(XCH):
                if rep == 0 and (rt, c) in xc_pre:
                    xc = xc_pre[(rt, c)]
                else:
                    xc = xp.tile([P, KB_CH * P], F32, tag="xc")
                    nc.sync.dma_start(
                        out=xc,
                        in_=xv[rt, :, c * (KB_CH * P):(c + 1) * (KB_CH * P)])
                xc3 = xc.rearrange("p (kb j) -> p kb j", j=P)
                sl = slice(c * KB_CH, (c + 1) * KB_CH)
                nc.vector.tensor_reduce(
                    out=am[:, sl], in_=xc3, axis=mybir.AxisListType.X,
                    op=mybir.AluOpType.max, apply_absolute_value=True)
                nc.vector.reciprocal(out=rc[:, sl], in_=am[:, sl])
                # s2b = bf16(am/224), r2 = 224/am on ACT; diag strip per chunk
                nc.scalar.activation(
                    out=s2b[:, sl], in_=am[:, sl],
                    func=mybir.ActivationFunctionType.Copy, scale=1.0 / 224.0)
                nc.scalar.activation(
                    out=r2[:, sl], in_=rc[:, sl],
                    func=mybir.ActivationFunctionType.Copy, scale=224.0)
                nc.gpsimd.affine_select(
                    out=diag[:, sl, :],
                    in_=s2b[:, sl, None].broadcast_to([P, KB_CH, P]),
                    compare_op=mybir.AluOpType.is_equal,
                    fill=0.0, base=0,
                    pattern=[[0, KB_CH], [-1, P]],
                    channel_multiplier=1)
                # quant-mul xq = x * (224/am), split DVE / GpSimd; the
                # last tile runs all on DVE (GpSimd's diag is the tail's
                # critical path, DVE idles there)
                for h in range(2):
                    hs = slice(h * (KB_CH // 2), (h + 1) * (KB_CH // 2))
                    ksl = slice(c * KB_CH + h * (KB_CH // 2),
                                c * KB_CH + (h + 1) * (KB_CH // 2))
                    dst = xqa[:, c] if h == 0 else xqb[:, c]
                    if h == 0 or rt == RT - 1:
                        nc.vector.scalar_tensor_tensor(
                            out=dst,
                            in0=xc3[:, hs, :],
                            scalar=224.0,
                            in1=rc[:, ksl, None].broadcast_to(
                                [P, KB_CH // 2, P]),
                            op0=mybir.AluOpType.mult,
                            op1=mybir.AluOpType.mult)
                    else:
                        nc.gpsimd.tensor_tensor(
                            out=dst,
                            in0=xc3[:, hs, :],
                            in1=r2[:, ksl, None].broadcast_to(
                                [P, KB_CH // 2, P]),
                            op=mybir.AluOpType.mult)

            # ---- transpose+dequant on PE, copy out per 4-block group ----
            xdt = tp.tile([P, KB, P], BF16, tag="xdt")
            for g in range(KB // G4):
                pt = pst.tile([P, G4, P], F32, tag="pt")
                for i in range(G4):
                    kb = g * G4 + i
                    c, j = kb // KB_CH, kb % KB_CH
                    lhs = (xqa[:, c, j, :] if j < KB_CH // 2
                           else xqb[:, c, j - KB_CH // 2, :])
                    nc.tensor.matmul(pt[:, i, :], lhsT=lhs,
                                     rhs=diag[:, kb, :], start=True, stop=True)
                nc.scalar.activation(
                    out=xdt[:, g * G4:(g + 1) * G4, :], in_=pt,
                    func=mybir.ActivationFunctionType.Copy)

            # ---- main GEMM: accumulate 56 k-blocks ----
            py = psy.tile([P, N], F32, tag="py")
            for kb in range(KB):
                nc.tensor.matmul(py, lhsT=xdt[:, kb, :], rhs=w[:, kb, :],
                                 start=(kb == 0), stop=(kb == KB - 1))
            y_sb = yp.tile([P, N], F32, tag="ysb")
            nc.scalar.activation(
                out=y_sb, in_=py, func=mybir.ActivationFunctionType.Copy)
            # issue the y store from the Scalar DGE queue: a sync-queue
            # dma_start here would stall all later x loads behind MM(rt)
            nc.scalar.dma_start(out=yv[rt], in_=y_sb)

    nc.compile()
    return nc


def prep_w(w_q: np.ndarray, w_s: np.ndarray) -> np.ndarray:
    Nb, Kb = w_s.shape
    w_dq = (w_q.reshape(Nb, P, Kb, P) * w_s[:, None, :, None]).reshape(N, K)
    w_t = np.ascontiguousarray(w_dq.T).astype(ml_dtypes.bfloat16)  # [K, N]
    # partition-major repack: [P, KB*N] with w_pk[p, kb, n] = w_t[kb*128+p, n]
    return np.ascontiguousarray(
        w_t.reshape(Kb, P, N).transpose(1, 0, 2).reshape(P, Kb * N))


def kernel(x: np.ndarray, w_q: np.ndarray, w_s: np.ndarray, _trace=False) -> np.ndarray:
    if "nc" not in _CACHE:
        _CACHE["nc"] = build_nc()
    nc = _CACHE["nc"]

    w_host = prep_w(np.asarray(w_q, np.float32), np.asarray(w_s, np.float32))
    xf = np.ascontiguousarray(np.asarray(x, np.float32).reshape(ROWS, K))
    in_maps = [
        {"x": np.ascontiguousarray(xf[c * R_CORE:(c + 1) * R_CORE]), "w": w_host}
        for c in range(NCORES)
    ]
    res = bass_utils.run_bass_kernel_spmd(
        nc, in_maps, core_ids=list(range(NCORES)), trace=_trace)
    y = np.concatenate([res.results[c]["y"] for c in range(NCORES)], axis=0)
    _CACHE["last_exec_time_ns"] = res.exec_time_ns
    return y.reshape(B, S, N)
